# revision 1
# baseline (speedup 1.0000x reference)
"""Distributed causal multi-head attention block on 8 TRN2 NeuronCores.

Tensor-parallel over heads (2 heads/core):
  - host: pre-cast to bf16, pre-transpose x -> xT [C, B*T], shard W_attn
    columns by head pair, permute W_proj rows to the AllToAll delivery order.
  - core i: computes qT,kT (transposed, W as lhsT) and v (natural, xT as
    lhsT) for its 2 heads, streaming xT k-chunk tiles from HBM.  Causal
    attention in transposed layout, head-OUTER: per 128-chunk of keys,
    sT = kT.T @ qT sliced to the causally-valid query range, exp on ScalarE
    (scale 1/8 folded in), diagonal 128x128 triangle masked on VectorE,
    av^T accumulated over key chunks with an extra ones column in v giving
    softmax row sums for free.  Per head, attention output is normalized and
    streamed straight into an AllToAll bounce; the h0 AllToAll overlaps the
    h1 attention, and the h1 AllToAll overlaps the first half of the output
    projection.  Each core projects its own 512-row t-shard with the full
    (row-permuted) W_proj; host concatenates the 8 shards.
"""

import numpy as np
import ml_dtypes

import concourse.bass as bass
import concourse.mybir as mybir
import concourse.tile as tile
from concourse import bacc
from concourse.bass_utils import run_bass_kernel_spmd

P = 128
B, T, C = 2, 2048, 1024
H, D = 16, 64
NCORES = 8
HPC = H // NCORES          # heads per core = 2
BT = B * T                 # 4096
TSH = BT // NCORES         # 512 rows per core shard
KC = C // P                # 8 contraction chunks
NBLK = BT // TSH           # 8 t-blocks of 512 (== rank blocks)
QW = 1024                  # query block width for attention
QB2 = T // QW              # 2 query blocks per batch
CH = T // P                # 16 key chunks of 128 per batch
F32 = mybir.dt.float32
BF16 = mybir.dt.bfloat16
SCALE = 1.0 / 8.0          # 1/sqrt(D)


def build_nc():
    nc = bacc.Bacc(None, target_bir_lowering=False)

    xT = nc.dram_tensor("xT", [C, BT], BF16, kind="ExternalInput")
    w_qk = nc.dram_tensor("w_qk", [C, 2 * P], BF16, kind="ExternalInput")
    w_v = nc.dram_tensor("w_v", [C, P], BF16, kind="ExternalInput")
    b_qk = nc.dram_tensor("b_qk", [2 * P], F32, kind="ExternalInput")
    b_v = nc.dram_tensor("b_v", [P], F32, kind="ExternalInput")
    w_pr = nc.dram_tensor("w_proj", [C, C], BF16, kind="ExternalInput")
    b_pr = nc.dram_tensor("b_proj", [C], F32, kind="ExternalInput")
    maskm = nc.dram_tensor("mask", [P, P], BF16, kind="ExternalInput")
    out = nc.dram_tensor("out", [TSH, C], F32, kind="ExternalOutput")

    with tile.TileContext(nc) as tc:
        with (
            tc.tile_pool(name="consts", bufs=1) as consts,
            tc.tile_pool(name="persist", bufs=1) as persist,
            tc.tile_pool(name="xtg", bufs=3) as xtg_pool,
            tc.tile_pool(name="pt", bufs=6) as pt_pool,
            tc.tile_pool(name="avs", bufs=3) as avs_pool,
            tc.tile_pool(name="rec", bufs=3) as rec_pool,
            tc.tile_pool(name="ps_a", bufs=2, space="PSUM") as ps_a,
            tc.tile_pool(name="ps_b", bufs=2, space="PSUM") as ps_b,
            tc.tile_pool(name="dram", bufs=1, space="DRAM") as dram,
            tc.tile_pool(name="dram_rec", bufs=4, space="DRAM") as dram_rec,
        ):
            # ---- constants to SBUF ----
            wqk_sb = consts.tile([P, KC, 2 * P], BF16)
            nc.sync.dma_start(wqk_sb[:], w_qk.ap().rearrange("(kc p) m -> p kc m", p=P))
            wv_sb = consts.tile([P, KC, P], BF16)
            nc.sync.dma_start(wv_sb[:], w_v.ap().rearrange("(kc p) m -> p kc m", p=P))
            wpr_sb = consts.tile([P, KC, C], BF16)
            nc.sync.dma_start(wpr_sb[:], w_pr.ap().rearrange("(kc p) m -> p kc m", p=P))
            bqk_sb = consts.tile([P, 2], F32)
            nc.sync.dma_start(bqk_sb[:], b_qk.ap().rearrange("(m p) -> p m", p=P))
            bv_sb = consts.tile([P, P], F32)
            nc.sync.dma_start(
                bv_sb[:],
                b_v.ap().rearrange("(o m) -> o m", o=1).to_broadcast((P, P)),
            )
            bpr_sb = consts.tile([P, C], F32)
            nc.sync.dma_start(
                bpr_sb[:],
                b_pr.ap().rearrange("(o m) -> o m", o=1).to_broadcast((P, C)),
            )
            mask_sb = consts.tile([P, P], BF16)
            nc.sync.dma_start(mask_sb[:], maskm.ap())

            # ---- phase 1: qT, kT (transposed) and v (natural) for my heads ----
            qkT = persist.tile([P, BT], BF16)   # rows: q cols h0|h1
            kT = persist.tile([P, BT], BF16)
            # vext layout: [tk-part, chunk, 130]: cols 0:64 v_h0, 64 ones,
            # 65:129 v_h1, 129 ones
            vext = persist.tile([P, BT // P, 130], BF16)
            nc.vector.memset(vext[:, :, 64], 1.0)
            nc.vector.memset(vext[:, :, 129], 1.0)

            xT_blocked = xT.ap().rearrange(
                "(kh kc p) (r t) -> r kh p kc t", p=P, r=NBLK, kh=2
            )
            for r in range(NBLK):
                xtg_r = xtg_pool.tile([P, KC, TSH], BF16)
                # split the 1MB block load so matmuls start at half-load
                nc.sync.dma_start(xtg_r[:, 0:KC // 2, :], xT_blocked[r, 0])
                nc.sync.dma_start(xtg_r[:, KC // 2:KC, :], xT_blocked[r, 1])
                # qT / kT together in one [128, 1024] psum (2 banks)
                ps = ps_a.tile([P, 2 * TSH], F32, name="ps_qk", tag="a")
                for m in range(2):
                    for kc in range(KC):
                        nc.tensor.matmul(
                            ps[:, m * TSH:(m + 1) * TSH],
                            lhsT=wqk_sb[:, kc, m * P:(m + 1) * P],
                            rhs=xtg_r[:, kc, :],
                            start=(kc == 0),
                            stop=(kc == KC - 1),
                        )
                for m, dst in ((0, qkT), (1, kT)):
                    nc.scalar.activation(
                        dst[:, r * TSH:(r + 1) * TSH],
                        ps[:, m * TSH:(m + 1) * TSH],
                        mybir.ActivationFunctionType.Identity,
                        bias=bqk_sb[:, m:m + 1], scale=1.0,
                    )
                # v natural: lhsT = xT chunk [c_in, t 128], rhs = W_v [c_in, 128]
                for mt in range(TSH // P):
                    psv = ps_b.tile([P, 2 * TSH], F32, name="ps_v", tag="b")
                    for kc in range(KC):
                        nc.tensor.matmul(
                            psv[:, 0:P],
                            lhsT=xtg_r[:, kc, mt * P:(mt + 1) * P],
                            rhs=wv_sb[:, kc, :],
                            start=(kc == 0),
                            stop=(kc == KC - 1),
                        )
                    ch = r * (TSH // P) + mt
                    for h in range(HPC):
                        nc.vector.tensor_tensor(
                            vext[:, ch, h * 65:h * 65 + 64],
                            psv[:, h * D:(h + 1) * D],
                            bv_sb[:, h * D:(h + 1) * D],
                            mybir.AluOpType.add,
                        )

            # ---- phase 2+3: causal attention (head-outer) + per-head AllToAll ----
            a2a_in = [
                dram.tile([NCORES * D, TSH], BF16, name=f"a2a_in_{h}")
                for h in range(HPC)
            ]
            a2a_out = [
                dram.tile([NCORES * D, TSH], BF16, name=f"a2a_out_{h}")
                for h in range(HPC)
            ]
            for h in range(HPC):
                for b in range(B):
                    for qb in range(QB2):
                        nch = (qb + 1) * (QW // P)
                        q0 = b * T + qb * QW
                        ps_o = ps_b.tile([P, QW], F32, name="ps_av", tag="b")
                        for c in range(nch):
                            j = c - qb * (QW // P)   # >=0: diagonal chunk index
                            lo = max(0, j * P)       # first causally-valid q col
                            ps = ps_a.tile([P, QW], F32, name="ps_s", tag="a")
                            for half in range(2):
                                s0 = max(lo, half * TSH)
                                s1 = (half + 1) * TSH
                                if s0 >= s1:
                                    continue
                                nc.tensor.matmul(
                                    ps[:, s0:s1],
                                    lhsT=kT[h * D:(h + 1) * D,
                                            b * T + c * P: b * T + (c + 1) * P],
                                    rhs=qkT[h * D:(h + 1) * D, q0 + s0: q0 + s1],
                                    start=True, stop=True,
                                )
                            pt = pt_pool.tile([P, QW], BF16)
                            nc.scalar.activation(
                                pt[:, lo:QW], ps[:, lo:QW],
                                mybir.ActivationFunctionType.Exp,
                                scale=SCALE,
                            )
                            if j >= 0:
                                # triangle mask on the first 128 valid cols
                                nc.vector.tensor_tensor(
                                    pt[:, lo:lo + P], pt[:, lo:lo + P],
                                    mask_sb[:],
                                    mybir.AluOpType.mult,
                                )
                            for half in range(2):
                                s0 = max(lo, half * TSH)
                                s1 = (half + 1) * TSH
                                if s0 >= s1:
                                    continue
                                nc.tensor.matmul(
                                    ps_o[:65, s0:s1],
                                    lhsT=vext[:, b * CH + c, h * 65:h * 65 + 65],
                                    rhs=pt[:, s0:s1],
                                    start=(c == 0), stop=(c == nch - 1),
                                )
                        # early copy frees the PSUM slot; normalize off-path
                        av_sb = avs_pool.tile([65, QW], F32)
                        nc.vector.tensor_copy(av_sb[:], ps_o[:65, :])
                        rec = rec_pool.tile([1, QW], F32)
                        nc.vector.reciprocal(rec[:], av_sb[64:65, :])
                        rec_dram = dram_rec.tile([1, QW], F32, name="rec_dram")
                        nc.sync.dma_start(rec_dram[:], rec[:])
                        rec_rep = rec_pool.tile([D, QW], F32, name="rec_rep")
                        nc.sync.dma_start(
                            rec_rep[:], rec_dram[0:1, :].to_broadcast((D, QW))
                        )
                        att_n = avs_pool.tile([D, QW], BF16, name="att_n")
                        nc.vector.tensor_tensor(
                            att_n[:], av_sb[0:64, :], rec_rep[:],
                            mybir.AluOpType.mult,
                        )
                        g0 = b * (T // TSH) + qb * (QW // TSH)
                        for half in range(2):
                            nc.sync.dma_start(
                                a2a_in[h][(g0 + half) * D:(g0 + half + 1) * D, :],
                                att_n[:, half * TSH:(half + 1) * TSH],
                            )
                nc.gpsimd.collective_compute(
                    "AllToAll",
                    mybir.AluOpType.bypass,
                    ins=[a2a_in[h].opt()],
                    outs=[a2a_out[h].opt()],
                    replica_groups=[list(range(NCORES))],
                )

            # ---- phase 4: output projection for my shard ----
            # A2A h out rows: [src-rank r x 64] = channels (r, h) — W_proj rows
            # were host-permuted to this order: kc chunk i<4 from h0, i>=4 h1.
            att_sb = [
                persist.tile([P, KC // 2, TSH], BF16, name=f"att_sb_{h}")
                for h in range(HPC)
            ]
            out_sb = persist.tile([P, TSH // P, C], F32)
            ps_pr = [
                pool.tile([P, 2 * TSH], F32, name=f"ps_pr_{i}", tag=t)
                for i, (pool, t) in enumerate(
                    [(ps_a, "a"), (ps_a, "a"), (ps_b, "b"), (ps_b, "b")]
                )
            ]
            for h in range(HPC):
                nc.sync.dma_start(
                    att_sb[h][:],
                    a2a_out[h].rearrange("(kc p) t -> p kc t", p=P),
                )
                for mt in range(TSH // P):
                    for nb in range(C // TSH):
                        ps = ps_pr[mt][:, nb * TSH:(nb + 1) * TSH]
                        for kc in range(KC // 2):
                            nc.tensor.matmul(
                                ps,
                                lhsT=att_sb[h][:, kc, mt * P:(mt + 1) * P],
                                rhs=wpr_sb[:, h * (KC // 2) + kc,
                                           nb * TSH:(nb + 1) * TSH],
                                start=(h == 0 and kc == 0),
                                stop=(h == HPC - 1 and kc == KC // 2 - 1),
                            )
                        if h == HPC - 1:
                            nc.vector.tensor_tensor(
                                out_sb[:, mt, nb * TSH:(nb + 1) * TSH],
                                ps,
                                bpr_sb[:, nb * TSH:(nb + 1) * TSH],
                                mybir.AluOpType.add,
                            )
            nc.sync.dma_start(
                out.ap().rearrange("(mt p) c -> p mt c", p=P), out_sb[:]
            )
    nc.finalize()
    return nc


_NC_CACHE = None


def _get_nc():
    global _NC_CACHE
    if _NC_CACHE is None:
        _NC_CACHE = build_nc()
    return _NC_CACHE


def make_in_maps(x, W_attn, b_attn, W_proj, b_proj):
    bf = ml_dtypes.bfloat16
    x_flat = np.asarray(x, np.float32).reshape(BT, C)
    xT_bf = np.ascontiguousarray(x_flat.T).astype(bf)
    W_attn = np.asarray(W_attn, np.float32)
    b_attn = np.asarray(b_attn, np.float32)
    b_proj = np.asarray(b_proj, np.float32)
    # permute W_proj rows to the split-A2A delivery order:
    # [r0h0 | r1h0 | ... | r7h0 | r0h1 | ... | r7h1]
    perm = np.concatenate(
        [np.arange(r * P + h * D, r * P + (h + 1) * D)
         for h in range(HPC) for r in range(NCORES)]
    )
    W_proj_bf = np.ascontiguousarray(
        np.asarray(W_proj, np.float32)[perm]
    ).astype(bf)
    mask = (np.arange(P)[None, :] >= np.arange(P)[:, None]).astype(bf)

    in_maps = []
    for i in range(NCORES):
        cs = slice(i * P, (i + 1) * P)
        w_qk = np.concatenate(
            [W_attn[:, 0:C][:, cs], W_attn[:, C:2 * C][:, cs]], axis=1
        ).astype(bf)
        b_qk = np.concatenate(
            [b_attn[0:C][cs], b_attn[C:2 * C][cs]]
        ).astype(np.float32)
        in_maps.append({
            "xT": xT_bf,
            "w_qk": np.ascontiguousarray(w_qk),
            "w_v": np.ascontiguousarray(W_attn[:, 2 * C:3 * C][:, cs]).astype(bf),
            "b_qk": np.ascontiguousarray(b_qk),
            "b_v": np.ascontiguousarray(b_attn[2 * C:3 * C][cs]).astype(np.float32),
            "w_proj": W_proj_bf,
            "b_proj": b_proj,
            "mask": mask,
        })
    return in_maps


def kernel(x, W_attn, b_attn, W_proj, b_proj):
    nc = _get_nc()
    in_maps = make_in_maps(x, W_attn, b_attn, W_proj, b_proj)
    res = run_bass_kernel_spmd(nc, in_maps, core_ids=list(range(NCORES)))
    shards = [np.asarray(res.results[i]["out"], np.float32) for i in range(NCORES)]
    return np.concatenate(shards, axis=0).reshape(B, T, C)



# revision 8
# speedup vs baseline: 1.1496x; 1.1496x over previous
"""Distributed causal multi-head attention block on 8 TRN2 NeuronCores.

Tensor-parallel over heads (2 heads/core):
  - host: pre-cast to bf16, pre-transpose x -> xT [C, B*T], shard W_attn
    columns by head pair, permute W_proj rows to the AllToAll delivery order.
  - core i: computes qT,kT (transposed, W as lhsT) and v (natural, xT as
    lhsT) for its 2 heads, streaming xT k-chunk tiles from HBM.  Causal
    attention in transposed layout, head-OUTER: per 128-chunk of keys,
    sT = kT.T @ qT sliced to the causally-valid query range, exp on ScalarE
    (scale 1/8 folded in), diagonal 128x128 triangle masked on VectorE,
    av^T accumulated over key chunks with an extra ones column in v giving
    softmax row sums for free.  Per head, attention output is normalized and
    streamed straight into an AllToAll bounce; the h0 AllToAll overlaps the
    h1 attention, and the h1 AllToAll overlaps the first half of the output
    projection.  Each core projects its own 512-row t-shard with the full
    (row-permuted) W_proj; host concatenates the 8 shards.
"""

import numpy as np
import ml_dtypes

import concourse.bass as bass
import concourse.mybir as mybir
import concourse.tile as tile
from concourse import bacc
from concourse.bass_utils import run_bass_kernel_spmd

P = 128
B, T, C = 2, 2048, 1024
H, D = 16, 64
NCORES = 8
HPC = H // NCORES          # heads per core = 2
BT = B * T                 # 4096
TSH = BT // NCORES         # 512 rows per core shard
KC = C // P                # 8 contraction chunks
NBLK = BT // TSH           # 8 t-blocks of 512 (== rank blocks)
QW = 1024                  # query block width for attention
QB2 = T // QW              # 2 query blocks per batch
CH = T // P                # 16 key chunks of 128 per batch
F32 = mybir.dt.float32
BF16 = mybir.dt.bfloat16
SCALE = 1.0 / 8.0          # 1/sqrt(D)


def build_nc():
    nc = bacc.Bacc(None, target_bir_lowering=False)

    xT = nc.dram_tensor("xT", [C, BT], BF16, kind="ExternalInput")
    w_qk = nc.dram_tensor("w_qk", [C, 2 * P], BF16, kind="ExternalInput")
    w_v = nc.dram_tensor("w_v", [C, P], BF16, kind="ExternalInput")
    b_qk = nc.dram_tensor("b_qk", [2 * P], F32, kind="ExternalInput")
    b_v = nc.dram_tensor("b_v", [P], F32, kind="ExternalInput")
    w_pr = nc.dram_tensor("w_proj", [C, C], BF16, kind="ExternalInput")
    b_pr = nc.dram_tensor("b_proj", [C], F32, kind="ExternalInput")
    maskm = nc.dram_tensor("mask", [P, P], BF16, kind="ExternalInput")
    out = nc.dram_tensor("out", [TSH, C], F32, kind="ExternalOutput")

    with tile.TileContext(nc) as tc:
        with (
            tc.tile_pool(name="consts", bufs=1) as consts,
            tc.tile_pool(name="persist", bufs=1) as persist,
            tc.tile_pool(name="xtg", bufs=3) as xtg_pool,
            tc.tile_pool(name="pt", bufs=6) as pt_pool,
            tc.tile_pool(name="avs", bufs=3) as avs_pool,
            tc.tile_pool(name="rec", bufs=3) as rec_pool,
            tc.tile_pool(name="ps_a", bufs=2, space="PSUM") as ps_a,
            tc.tile_pool(name="ps_b", bufs=2, space="PSUM") as ps_b,
            tc.tile_pool(name="dram", bufs=1, space="DRAM") as dram,
        ):
            # ---- first xT block + small constants first (critical path); the
            # big W_proj load and a tiny collective-warmup are issued later ----
            xT_blocked = xT.ap().rearrange(
                "(kh kc p) (r t) -> r kh p kc t", p=P, r=NBLK, kh=2
            )
            xtg_first = xtg_pool.tile([P, KC, TSH], BF16, name="xtg_first")
            nc.sync.dma_start(xtg_first[:, 0:KC // 2, :], xT_blocked[0, 0])
            nc.sync.dma_start(xtg_first[:, KC // 2:KC, :], xT_blocked[0, 1])
            wqk_sb = consts.tile([P, KC, 2 * P], BF16)
            nc.sync.dma_start(wqk_sb[:], w_qk.ap().rearrange("(kc p) m -> p kc m", p=P))
            wv_sb = consts.tile([P, KC, P], BF16)
            nc.sync.dma_start(wv_sb[:], w_v.ap().rearrange("(kc p) m -> p kc m", p=P))
            bqk_sb = consts.tile([P, 2], F32)
            nc.sync.dma_start(bqk_sb[:], b_qk.ap().rearrange("(m p) -> p m", p=P))
            bv_sb = consts.tile([P, P], F32)
            nc.sync.dma_start(
                bv_sb[:],
                b_v.ap().rearrange("(o m) -> o m", o=1).to_broadcast((P, P)),
            )
            mask_sb = consts.tile([P, P], BF16)
            nc.sync.dma_start(mask_sb[:], maskm.ap())

            # tiny AllToAll to absorb the first-collective ncfw warmup latency;
            # runs during phase 1, result unused
            warm_sb = consts.tile([1, 2 * NCORES], BF16)
            nc.vector.memset(warm_sb[:], 0.0)
            warm_in = dram.tile([NCORES, 2], BF16, name="warm_in")
            warm_out = dram.tile([NCORES, 2], BF16, name="warm_out")
            nc.sync.dma_start(
                warm_in.rearrange("(o r) m -> o (r m)", o=1), warm_sb[:]
            )
            nc.gpsimd.collective_compute(
                "AllToAll",
                mybir.AluOpType.bypass,
                ins=[warm_in.opt()],
                outs=[warm_out.opt()],
                replica_groups=[list(range(NCORES))],
            )

            # ---- phase 1: qT, kT (transposed) and v (natural) for my heads ----
            qkT = persist.tile([P, BT], BF16)   # rows: q cols h0|h1
            kT = persist.tile([P, BT], BF16)
            # vext layout: [tk-part, chunk, 130]: cols 0:64 v_h0, 64 ones,
            # 65:129 v_h1, 129 ones
            vext = persist.tile([P, BT // P, 130], BF16)
            nc.vector.memset(vext[:, :, 64], 1.0)
            nc.vector.memset(vext[:, :, 129], 1.0)

            for r in range(NBLK):
                if r == 0:
                    xtg_r = xtg_first
                else:
                    xtg_r = xtg_pool.tile([P, KC, TSH], BF16)
                    # split the 1MB block load so matmuls start at half-load
                    nc.sync.dma_start(xtg_r[:, 0:KC // 2, :], xT_blocked[r, 0])
                    nc.sync.dma_start(xtg_r[:, KC // 2:KC, :], xT_blocked[r, 1])
                # qT / kT together in one [128, 1024] psum (2 banks)
                ps = ps_a.tile([P, 2 * TSH], F32, name="ps_qk", tag="a")
                for m in range(2):
                    for kc in range(KC):
                        nc.tensor.matmul(
                            ps[:, m * TSH:(m + 1) * TSH],
                            lhsT=wqk_sb[:, kc, m * P:(m + 1) * P],
                            rhs=xtg_r[:, kc, :],
                            start=(kc == 0),
                            stop=(kc == KC - 1),
                        )
                for m, dst in ((0, qkT), (1, kT)):
                    nc.scalar.activation(
                        dst[:, r * TSH:(r + 1) * TSH],
                        ps[:, m * TSH:(m + 1) * TSH],
                        mybir.ActivationFunctionType.Identity,
                        bias=bqk_sb[:, m:m + 1], scale=1.0,
                    )
                # v natural: lhsT = xT chunk [c_in, t 128], rhs = W_v [c_in, 128]
                for mt in range(TSH // P):
                    psv = ps_b.tile([P, 2 * TSH], F32, name="ps_v", tag="b")
                    for kc in range(KC):
                        nc.tensor.matmul(
                            psv[:, 0:P],
                            lhsT=xtg_r[:, kc, mt * P:(mt + 1) * P],
                            rhs=wv_sb[:, kc, :],
                            start=(kc == 0),
                            stop=(kc == KC - 1),
                        )
                    ch = r * (TSH // P) + mt
                    for h in range(HPC):
                        nc.vector.tensor_tensor(
                            vext[:, ch, h * 65:h * 65 + 64],
                            psv[:, h * D:(h + 1) * D],
                            bv_sb[:, h * D:(h + 1) * D],
                            mybir.AluOpType.add,
                        )

            # proj weights issued after the xT stream so they don't delay it
            wpr_sb = consts.tile([P, KC, C], BF16)
            nc.sync.dma_start(wpr_sb[:], w_pr.ap().rearrange("(kc p) m -> p kc m", p=P))
            bpr_sb = consts.tile([P, C], F32)
            nc.sync.dma_start(
                bpr_sb[:],
                b_pr.ap().rearrange("(o m) -> o m", o=1).to_broadcast((P, C)),
            )

            # ---- phase 2+3: causal attention (head-outer) + per-head AllToAll ----
            a2a_in = [
                dram.tile([NCORES * D, TSH], BF16, name=f"a2a_in_{h}")
                for h in range(HPC)
            ]
            a2a_out = [
                dram.tile([NCORES * D, TSH], BF16, name=f"a2a_out_{h}")
                for h in range(HPC)
            ]
            for h in range(HPC):
                for b in range(B):
                    for qb in range(QB2):
                        nch = (qb + 1) * (QW // P)
                        q0 = b * T + qb * QW
                        ps_o = ps_b.tile([P, QW], F32, name="ps_av", tag="b")
                        for c in range(nch):
                            j = c - qb * (QW // P)   # >=0: diagonal chunk index
                            lo = max(0, j * P)       # first causally-valid q col
                            ps = ps_a.tile([P, QW], F32, name="ps_s", tag="a")
                            for half in range(2):
                                s0 = max(lo, half * TSH)
                                s1 = (half + 1) * TSH
                                if s0 >= s1:
                                    continue
                                nc.tensor.matmul(
                                    ps[:, s0:s1],
                                    lhsT=kT[h * D:(h + 1) * D,
                                            b * T + c * P: b * T + (c + 1) * P],
                                    rhs=qkT[h * D:(h + 1) * D, q0 + s0: q0 + s1],
                                    start=True, stop=True,
                                )
                            pt = pt_pool.tile([P, QW], BF16)
                            nc.scalar.activation(
                                pt[:, lo:QW], ps[:, lo:QW],
                                mybir.ActivationFunctionType.Exp,
                                scale=SCALE,
                            )
                            if j >= 0:
                                # triangle mask on the first 128 valid cols
                                nc.vector.tensor_tensor(
                                    pt[:, lo:lo + P], pt[:, lo:lo + P],
                                    mask_sb[:],
                                    mybir.AluOpType.mult,
                                )
                            for half in range(2):
                                s0 = max(lo, half * TSH)
                                s1 = (half + 1) * TSH
                                if s0 >= s1:
                                    continue
                                nc.tensor.matmul(
                                    ps_o[:65, s0:s1],
                                    lhsT=vext[:, b * CH + c, h * 65:h * 65 + 65],
                                    rhs=pt[:, s0:s1],
                                    start=(c == 0), stop=(c == nch - 1),
                                )
                        # normalize: fast-approx reciprocal of the row-sum row
                        # (custom DVE ops can't read PSUM - copy to SBUF first),
                        # broadcast across the 64 head dims on GpSimd, then
                        # scale straight from PSUM
                        den = rec_pool.tile([1, QW], F32, name="den")
                        nc.vector.tensor_copy(den[:], ps_o[64:65, :])
                        rec = rec_pool.tile([1, QW], F32)
                        nc.vector.reciprocal_approx_fast(rec[:], den[:])
                        rec_rep = rec_pool.tile([D, QW], F32, name="rec_rep")
                        nc.gpsimd.partition_broadcast(rec_rep[:], rec[:], channels=D)
                        att_n = avs_pool.tile([D, QW], BF16, name="att_n")
                        nc.vector.tensor_tensor(
                            att_n[:], ps_o[0:64, :], rec_rep[:],
                            mybir.AluOpType.mult,
                        )
                        g0 = b * (T // TSH) + qb * (QW // TSH)
                        for half in range(2):
                            nc.sync.dma_start(
                                a2a_in[h][(g0 + half) * D:(g0 + half + 1) * D, :],
                                att_n[:, half * TSH:(half + 1) * TSH],
                            )
                nc.gpsimd.collective_compute(
                    "AllToAll",
                    mybir.AluOpType.bypass,
                    ins=[a2a_in[h].opt()],
                    outs=[a2a_out[h].opt()],
                    replica_groups=[list(range(NCORES))],
                )

            # ---- phase 4: output projection for my shard ----
            # A2A h out rows: [src-rank r x 64] = channels (r, h) — W_proj rows
            # were host-permuted to this order: kc chunk i<4 from h0, i>=4 h1.
            att_sb = [
                persist.tile([P, KC // 2, TSH], BF16, name=f"att_sb_{h}")
                for h in range(HPC)
            ]
            out_sb = persist.tile([P, TSH // P, C], F32)
            ps_pr = [
                pool.tile([P, 2 * TSH], F32, name=f"ps_pr_{i}", tag=t)
                for i, (pool, t) in enumerate(
                    [(ps_a, "a"), (ps_a, "a"), (ps_b, "b"), (ps_b, "b")]
                )
            ]
            out_blocked = out.ap().rearrange("(mt p) c -> mt p c", p=P)
            for h in range(HPC):
                nc.sync.dma_start(
                    att_sb[h][:],
                    a2a_out[h].rearrange("(kc p) t -> p kc t", p=P),
                )
                for mt in range(TSH // P):
                    for nb in range(C // TSH):
                        ps = ps_pr[mt][:, nb * TSH:(nb + 1) * TSH]
                        for kc in range(KC // 2):
                            nc.tensor.matmul(
                                ps,
                                lhsT=att_sb[h][:, kc, mt * P:(mt + 1) * P],
                                rhs=wpr_sb[:, h * (KC // 2) + kc,
                                           nb * TSH:(nb + 1) * TSH],
                                start=(h == 0 and kc == 0),
                                stop=(h == HPC - 1 and kc == KC // 2 - 1),
                            )
                        if h == HPC - 1:
                            nc.vector.tensor_tensor(
                                out_sb[:, mt, nb * TSH:(nb + 1) * TSH],
                                ps,
                                bpr_sb[:, nb * TSH:(nb + 1) * TSH],
                                mybir.AluOpType.add,
                            )
                    if h == HPC - 1:
                        # stream each 128-row output block as soon as ready
                        nc.sync.dma_start(out_blocked[mt], out_sb[:, mt, :])
    nc.finalize()
    return nc


_NC_CACHE = None


def _get_nc():
    global _NC_CACHE
    if _NC_CACHE is None:
        _NC_CACHE = build_nc()
    return _NC_CACHE


def make_in_maps(x, W_attn, b_attn, W_proj, b_proj):
    bf = ml_dtypes.bfloat16
    x_flat = np.asarray(x, np.float32).reshape(BT, C)
    xT_bf = np.ascontiguousarray(x_flat.T).astype(bf)
    W_attn = np.asarray(W_attn, np.float32)
    b_attn = np.asarray(b_attn, np.float32)
    b_proj = np.asarray(b_proj, np.float32)
    # permute W_proj rows to the split-A2A delivery order:
    # [r0h0 | r1h0 | ... | r7h0 | r0h1 | ... | r7h1]
    perm = np.concatenate(
        [np.arange(r * P + h * D, r * P + (h + 1) * D)
         for h in range(HPC) for r in range(NCORES)]
    )
    W_proj_bf = np.ascontiguousarray(
        np.asarray(W_proj, np.float32)[perm]
    ).astype(bf)
    mask = (np.arange(P)[None, :] >= np.arange(P)[:, None]).astype(bf)

    in_maps = []
    for i in range(NCORES):
        cs = slice(i * P, (i + 1) * P)
        w_qk = np.concatenate(
            [W_attn[:, 0:C][:, cs], W_attn[:, C:2 * C][:, cs]], axis=1
        ).astype(bf)
        b_qk = np.concatenate(
            [b_attn[0:C][cs], b_attn[C:2 * C][cs]]
        ).astype(np.float32)
        in_maps.append({
            "xT": xT_bf,
            "w_qk": np.ascontiguousarray(w_qk),
            "w_v": np.ascontiguousarray(W_attn[:, 2 * C:3 * C][:, cs]).astype(bf),
            "b_qk": np.ascontiguousarray(b_qk),
            "b_v": np.ascontiguousarray(b_attn[2 * C:3 * C][cs]).astype(np.float32),
            "w_proj": W_proj_bf,
            "b_proj": b_proj,
            "mask": mask,
        })
    return in_maps


def kernel(x, W_attn, b_attn, W_proj, b_proj):
    nc = _get_nc()
    in_maps = make_in_maps(x, W_attn, b_attn, W_proj, b_proj)
    res = run_bass_kernel_spmd(nc, in_maps, core_ids=list(range(NCORES)))
    shards = [np.asarray(res.results[i]["out"], np.float32) for i in range(NCORES)]
    return np.concatenate(shards, axis=0).reshape(B, T, C)

